# revision 1
# baseline (speedup 1.0000x reference)
"""Trainium2 Bass kernel for MinimalLinearAttention.

  q = relu(x @ q_w.T + q_b); k = relu(x @ k_w.T + k_b); v = x @ v_w.T + v_b
  kv[b,h] = sum_s k[b,s,h,:] outer v[b,s,h,:]          (per batch, all tokens)
  out[b,s,h] = q[b,s,h,:] @ kv[b,h]
  y = out @ o_w.T + o_b

Sharding: token-parallel over 8 cores. Each core takes a 512-token slice of
every batch (2048 tokens), computes k/v projections + partial kv, AllReduces
kv across cores (per batch, overlapped with the q projection), then does the
q readout + output projection for its own tokens. Host concatenates slices.

Matmuls run as float32r (TF32-like single-pass, 1 cyc/row at N>=256) with
fp32 PSUM accumulation. Walrus requires float32r matmul operands to be
produced by a rounding instruction, so PE-feeding tiles are allocated as
float32r: PSUM evictions (ACT/DVE) round for free; DMA-loaded tensors get
one DVE conversion copy.

On-device layouts (per core):
  xt   = x_slice.T            [D=1024, T=2048]   (T cols batch-major: b*512+s)
  wq/wk/wv/wo = W.T           [Din=1024, Dout=1024]
  K, V                        [T, D]     (from xt-stationary matmuls)
  Q^T                         [D, T]     (from w-stationary matmuls)
  kv per (batch, head-pair)   [128, 128] block-diagonal (2 heads of 64)
  O^T                         [D, T]
  y                           [T, D]
"""

import os
import sys

os.environ.setdefault("MYCRO_LOCAL_CACHE", "1")

for _p in ("/opt/trn_rl_repo", "/root/.axon_site/_ro/trn_rl_repo"):
    if os.path.isdir(_p) and _p not in sys.path:
        sys.path.insert(0, _p)

import numpy as np

B, S, D, H, HD = 4, 4096, 1024, 16, 64
NCORES = 8
SC = S // NCORES          # 512 tokens per core per batch
T = B * SC                # 2048 rows per core
NPAIR = 8                 # head pairs (2 heads of 64 dims = 128 partitions)
NDIN = D // 128           # 8 Din tiles
NT = T // 128             # 16 T tiles per core
NTB = SC // 128           # 4 T tiles per batch

_CACHE = {}


def build_program():
    """f32r variant (v2): fast-start DMA order, wq prefetch, per-batch
    fused output projection, diagonal-packed kv collectives."""
    if "nc_f32r" in _CACHE:
        return _CACHE["nc_f32r"]

    import concourse.bacc as bacc
    import concourse.tile as tile
    from concourse import bass, mybir

    f32 = mybir.dt.float32
    FR = mybir.dt.float32r
    RELU = mybir.ActivationFunctionType.Relu

    nc = bacc.Bacc("TRN2", target_bir_lowering=False, debug=False,
                   num_devices=NCORES)

    xt_d = nc.dram_tensor("xt", [D, T], f32, kind="ExternalInput").ap()
    wq_d = nc.dram_tensor("wq", [D, D], f32, kind="ExternalInput").ap()
    wk_d = nc.dram_tensor("wk", [D, D], f32, kind="ExternalInput").ap()
    wv_d = nc.dram_tensor("wv", [D, D], f32, kind="ExternalInput").ap()
    wo_d = nc.dram_tensor("wo", [D, D], f32, kind="ExternalInput").ap()
    bq_d = nc.dram_tensor("bq", [128, NDIN], f32, kind="ExternalInput").ap()
    bk_d = nc.dram_tensor("bk", [1, D], f32, kind="ExternalInput").ap()
    bv_d = nc.dram_tensor("bv", [1, D], f32, kind="ExternalInput").ap()
    bo_d = nc.dram_tensor("bo", [1, D], f32, kind="ExternalInput").ap()
    y_d = nc.dram_tensor("y", [T, D], f32, kind="ExternalOutput").ap()

    HPB = 16 * 64  # bounce rows per batch: 16 heads x 64 d-rows

    from contextlib import ExitStack

    with tile.TileContext(nc) as tc:
        with ExitStack() as top:
            constp = top.enter_context(tc.tile_pool(name="const", bufs=1))
            rawp = top.enter_context(tc.tile_pool(name="raw", bufs=3))
            dramp = top.enter_context(
                tc.tile_pool(name="dram", bufs=1, space="DRAM"))
            psp = top.enter_context(
                tc.tile_pool(name="ps", bufs=3, space="PSUM"))
            pskvp = top.enter_context(
                tc.tile_pool(name="pskv", bufs=4, space="PSUM"))

            def load_fr(pool, dram_ap, shape, tag, name):
                raw = rawp.tile(shape, f32, tag="raw", name=f"raw_{name}")
                nc.sync.dma_start(raw[:], dram_ap)
                t = pool.tile(shape, FR, tag=tag, name=name)
                nc.vector.tensor_copy(t[:], raw[:])
                return t

            ones_raw = constp.tile([1, 128], f32, tag="ones_raw")
            nc.vector.memset(ones_raw[:], 1.0)
            ones = constp.tile([1, 128], FR, tag="ones")
            nc.vector.tensor_copy(ones[:], ones_raw[:])
            bq_sb = constp.tile([128, NDIN], f32, tag="bq")
            nc.sync.dma_start(bq_sb[:], bq_d[:])

            bnc_in = [dramp.tile([HPB, 64], f32, tag=f"bi{b}",
                                 name=f"bnc_in{b}") for b in range(B)]
            bnc_out = [dramp.tile([HPB, 64], f32, tag=f"bo{b}",
                                  addr_space="Shared", name=f"bnc_out{b}")
                       for b in range(B)]

            with tc.tile_pool(name="xtp", bufs=1) as xtp:
                # xt first chunks (cols 0:1024) + wk: minimal set for the
                # first matmuls; then wv, then xt second chunks
                xts = []
                for dn in range(NDIN):
                    t = xtp.tile([128, T], FR, tag=f"xt{dn}",
                                 name=f"xt_sb{dn}")
                    raw = rawp.tile([128, 1024], f32, tag="raw",
                                    name=f"raw_xt{dn}_0")
                    nc.sync.dma_start(raw[:], xt_d[dn * 128:(dn + 1) * 128,
                                                   0:1024])
                    nc.vector.tensor_copy(t[:, 0:1024], raw[:])
                    xts.append(t)

                with tc.tile_pool(name="wqpre", bufs=1) as wqprep, \
                        ExitStack() as st1:
                    wkvp = st1.enter_context(tc.tile_pool(name="wkv", bufs=1))
                    kvbp = st1.enter_context(tc.tile_pool(name="kvb", bufs=3))
                    kvexp = st1.enter_context(tc.tile_pool(name="kvex", bufs=8))
                    wk_sb = [load_fr(wkvp, wk_d[dn * 128:(dn + 1) * 128, :],
                                     [128, D], f"wk{dn}", f"wk_sb{dn}")
                             for dn in range(NDIN)]
                    bk_sb = load_fr(wkvp, bk_d[:], [1, D], "bk", "bk_sb")
                    wv_sb = [load_fr(wkvp, wv_d[dn * 128:(dn + 1) * 128, :],
                                     [128, D], f"wv{dn}", f"wv_sb{dn}")
                             for dn in range(NDIN)]
                    bv_sb = load_fr(wkvp, bv_d[:], [1, D], "bv", "bv_sb")
                    for dn in range(NDIN):
                        raw = rawp.tile([128, 1024], f32, tag="raw",
                                        name=f"raw_xt{dn}_1")
                        nc.sync.dma_start(
                            raw[:], xt_d[dn * 128:(dn + 1) * 128, 1024:2048])
                        nc.vector.tensor_copy(xts[dn][:, 1024:2048], raw[:])

                    wq_sb = [None] * NDIN

                    for b in range(B):
                        kvps = [pskvp.tile([128, 512], f32, tag="kvps",
                                           name=f"kvps{b}_{w}")
                                for w in range(4)]
                        for t in range(NTB):
                            gt = b * NTB + t
                            kt = kvbp.tile([128, D], FR, tag="kb")
                            vt = kvbp.tile([128, D], FR, tag="vb")
                            for w_sb, brow, dst, act in (
                                (wk_sb, bk_sb, kt, "relu"),
                                (wv_sb, bv_sb, vt, "copy"),
                            ):
                                for hf in range(2):
                                    ps = psp.tile([128, 512], f32, tag="ps")
                                    for dn in range(NDIN):
                                        nc.tensor.matmul(
                                            ps[:],
                                            xts[dn][:, gt * 128:(gt + 1) * 128],
                                            w_sb[dn][:, hf * 512:(hf + 1) * 512],
                                            start=(dn == 0), stop=False)
                                    nc.tensor.matmul(
                                        ps[:], ones[:, 0:128],
                                        brow[:, hf * 512:(hf + 1) * 512],
                                        start=False, stop=True)
                                    dsl = dst[:, hf * 512:(hf + 1) * 512]
                                    if act == "relu":
                                        nc.scalar.activation(dsl, ps[:], RELU)
                                    else:
                                        nc.vector.tensor_copy(dsl, ps[:])
                            for p in range(NPAIR):
                                nc.tensor.matmul(
                                    kvps[p // 2][:, (p % 2) * 256:(p % 2) * 256 + 256],
                                    kt[:, p * 128:(p + 1) * 128],
                                    vt[:, (p // 2) * 256:(p // 2) * 256 + 256],
                                    start=(t == 0 and p % 2 == 0),
                                    stop=(t == NTB - 1 and p % 2 == 1))
                        # ship only diagonal [64,64] blocks (head h = 2p+j)
                        for p in range(NPAIR):
                            for j in range(2):
                                ex = kvexp.tile([64, 64], f32, tag="kvex",
                                                name=f"kvex{b}_{p}_{j}")
                                nc.vector.tensor_copy(
                                    ex[:],
                                    kvps[p // 2][j * 64:(j + 1) * 64,
                                                 (p % 2) * 384 + j * 64:
                                                 (p % 2) * 384 + j * 64 + 64])
                                h = 2 * p + j
                                nc.sync.dma_start(
                                    bnc_in[b][h * 64:(h + 1) * 64, :], ex[:])
                        nc.gpsimd.collective_compute(
                            "AllReduce", mybir.AluOpType.add,
                            replica_groups=[list(range(NCORES))],
                            ins=[bnc_in[b].opt()], outs=[bnc_out[b].opt()])
                        if b == 1:
                            # prefetch the first wq tiles into spare SBUF so
                            # stage 2 starts without a weight-load bubble
                            for dn in range(2):
                                wq_sb[dn] = load_fr(
                                    wqprep, wq_d[dn * 128:(dn + 1) * 128, :],
                                    [128, D], f"wqp{dn}", f"wq_sb{dn}")

                    # ---- Stage 2: Q^T proj + readout + fused o-proj ----
                    st1.close()
                    with ExitStack() as st2:
                        wq2p = st2.enter_context(
                            tc.tile_pool(name="wq2", bufs=1))
                        wop = st2.enter_context(tc.tile_pool(name="wo", bufs=1))
                        otbp = st2.enter_context(
                            tc.tile_pool(name="otb", bufs=1))
                        qtp = st2.enter_context(tc.tile_pool(name="qt", bufs=3))
                        kvrawp = st2.enter_context(
                            tc.tile_pool(name="kvraw", bufs=4))
                        kvsbp = st2.enter_context(
                            tc.tile_pool(name="kvsb", bufs=8))
                        ytp = st2.enter_context(tc.tile_pool(name="yt", bufs=3))
                        for dn in range(2, NDIN):
                            wq_sb[dn] = load_fr(
                                wq2p, wq_d[dn * 128:(dn + 1) * 128, :],
                                [128, D], f"wq{dn}", f"wq_sb{dn}")
                        wo_sb = [load_fr(wop, wo_d[dn * 128:(dn + 1) * 128, :],
                                         [128, D], f"wo{dn}", f"wo_sb{dn}")
                                 for dn in range(NDIN)]
                        bo_sb = load_fr(wop, bo_d[:], [1, D], "bo", "bo_sb")

                        for b in range(B):
                            otb = [otbp.tile([128, 512], FR, tag=f"otb{p}",
                                             name=f"otb{b}_{p}")
                                   for p in range(NPAIR)]
                            for p in range(NPAIR):
                                raw = kvrawp.tile([128, 128], f32, tag="kvraw",
                                                  name=f"kvraw{p}_{b}")
                                nc.vector.memset(raw[:], 0.0)
                                for j in range(2):
                                    h = 2 * p + j
                                    nc.sync.dma_start(
                                        raw[j * 64:(j + 1) * 64,
                                            j * 64:(j + 1) * 64],
                                        bnc_out[b][h * 64:(h + 1) * 64, :])
                                kvsb = kvsbp.tile([128, 128], FR, tag="kvsb",
                                                  name=f"kvsb{p}_{b}")
                                nc.vector.tensor_copy(kvsb[:], raw[:])
                                ps = psp.tile([128, 512], f32, tag="ps")
                                for dn in range(NDIN):
                                    nc.tensor.matmul(
                                        ps[:],
                                        wq_sb[dn][:, p * 128:(p + 1) * 128],
                                        xts[dn][:, b * 512:(b + 1) * 512],
                                        start=(dn == 0), stop=(dn == NDIN - 1))
                                qt = qtp.tile([128, 512], FR, tag="qt")
                                nc.scalar.activation(qt[:], ps[:], RELU,
                                                     bias=bq_sb[:, p:p + 1])
                                pso = psp.tile([128, 512], f32, tag="ps")
                                nc.tensor.matmul(pso[:], kvsb[:], qt[:],
                                                 start=True, stop=True)
                                nc.vector.tensor_copy(otb[p][:], pso[:])
                            # fused output projection for this batch
                            for t in range(NTB):
                                gt = b * NTB + t
                                yt = ytp.tile([128, D], f32, tag="yt")
                                for hf in range(2):
                                    ps = psp.tile([128, 512], f32, tag="ps")
                                    for dn in range(NDIN):
                                        nc.tensor.matmul(
                                            ps[:],
                                            otb[dn][:, t * 128:(t + 1) * 128],
                                            wo_sb[dn][:, hf * 512:(hf + 1) * 512],
                                            start=(dn == 0), stop=False)
                                    nc.tensor.matmul(
                                        ps[:], ones[:, 0:128],
                                        bo_sb[:, hf * 512:(hf + 1) * 512],
                                        start=False, stop=True)
                                    nc.vector.tensor_copy(
                                        yt[:, hf * 512:(hf + 1) * 512], ps[:])
                                nc.sync.dma_start(
                                    y_d[gt * 128:(gt + 1) * 128, :], yt[:])

    nc.compile()
    _CACHE["nc_f32r"] = nc
    return nc


def build_program_bf16():
    """bf16 variant: all matmul operands bf16 (host-cast), flat SBUF layout
    with every weight resident, DMA ordering for fast PE start, and
    diagonal-packed kv collectives."""
    if "nc_bf16" in _CACHE:
        return _CACHE["nc_bf16"]

    import concourse.bacc as bacc
    import concourse.tile as tile
    from concourse import bass, mybir

    f32 = mybir.dt.float32
    BF = mybir.dt.bfloat16
    RELU = mybir.ActivationFunctionType.Relu

    nc = bacc.Bacc("TRN2", target_bir_lowering=False, debug=False,
                   num_devices=NCORES)

    xt_d = nc.dram_tensor("xt", [D, T], BF, kind="ExternalInput").ap()
    wq_d = nc.dram_tensor("wq", [D, D], BF, kind="ExternalInput").ap()
    wk_d = nc.dram_tensor("wk", [D, D], BF, kind="ExternalInput").ap()
    wv_d = nc.dram_tensor("wv", [D, D], BF, kind="ExternalInput").ap()
    wo_d = nc.dram_tensor("wo", [D, D], BF, kind="ExternalInput").ap()
    bq_d = nc.dram_tensor("bq", [128, NDIN], f32, kind="ExternalInput").ap()
    bk_d = nc.dram_tensor("bk", [1, D], BF, kind="ExternalInput").ap()
    bv_d = nc.dram_tensor("bv", [1, D], BF, kind="ExternalInput").ap()
    bo_d = nc.dram_tensor("bo", [1, D], BF, kind="ExternalInput").ap()
    y_d = nc.dram_tensor("y", [T, D], f32, kind="ExternalOutput").ap()

    HPB = 16 * 64  # bounce rows per batch: 16 heads x 64 d-rows

    with tile.TileContext(nc) as tc:
        with (
            tc.tile_pool(name="const", bufs=1) as constp,
            tc.tile_pool(name="wp", bufs=1) as wp,
            tc.tile_pool(name="xtp", bufs=1) as xtp,
            tc.tile_pool(name="otp", bufs=1) as otp,
            tc.tile_pool(name="kvb", bufs=3) as kvbp,
            tc.tile_pool(name="qt", bufs=4) as qtp,
            tc.tile_pool(name="kvex", bufs=8) as kvexp,
            tc.tile_pool(name="kvraw", bufs=4) as kvrawp,
            tc.tile_pool(name="kvsb", bufs=8) as kvsbp,
            tc.tile_pool(name="yt", bufs=3) as ytp,
            tc.tile_pool(name="dram", bufs=1, space="DRAM") as dramp,
            tc.tile_pool(name="ps", bufs=3, space="PSUM") as psp,
            tc.tile_pool(name="pskv", bufs=4, space="PSUM") as pskvp,
        ):
            # load order = scheduling priority: xt + wk first so the PE can
            # start, then wv, then wq/wo for the later stages
            xts = []
            for dn in range(NDIN):
                t = xtp.tile([128, T], BF, tag=f"xt{dn}", name=f"xt_sb{dn}")
                nc.sync.dma_start(t[:], xt_d[dn * 128:(dn + 1) * 128, :])
                xts.append(t)

            def loadw(dram_ap, tag):
                w = []
                for dn in range(NDIN):
                    t = wp.tile([128, D], BF, tag=f"{tag}{dn}",
                                name=f"{tag}_sb{dn}")
                    nc.sync.dma_start(t[:], dram_ap[dn * 128:(dn + 1) * 128, :])
                    w.append(t)
                return w

            wk_sb = loadw(wk_d, "wk")
            ones = constp.tile([1, 128], BF, tag="ones")
            nc.vector.memset(ones[:], 1.0)
            bk_sb = constp.tile([1, D], BF, tag="bk")
            nc.sync.dma_start(bk_sb[:], bk_d[:])
            wv_sb = loadw(wv_d, "wv")
            bv_sb = constp.tile([1, D], BF, tag="bv")
            nc.sync.dma_start(bv_sb[:], bv_d[:])
            wq_sb = loadw(wq_d, "wq")
            bq_sb = constp.tile([128, NDIN], f32, tag="bq")
            nc.sync.dma_start(bq_sb[:], bq_d[:])
            wo_sb = loadw(wo_d, "wo")
            bo_sb = constp.tile([1, D], BF, tag="bo")
            nc.sync.dma_start(bo_sb[:], bo_d[:])

            bnc_in = [dramp.tile([HPB, 64], f32, tag=f"bi{b}",
                                 name=f"bnc_in{b}") for b in range(B)]
            bnc_out = [dramp.tile([HPB, 64], f32, tag=f"bo{b}",
                                  addr_space="Shared", name=f"bnc_out{b}")
                       for b in range(B)]

            # ---- Stage 1: K,V projections + per-batch partial kv ----
            for b in range(B):
                kvps = [pskvp.tile([128, 512], f32, tag="kvps",
                                   name=f"kvps{b}_{w}") for w in range(4)]
                for t in range(NTB):
                    gt = b * NTB + t
                    kt = kvbp.tile([128, D], BF, tag="kb")
                    vt = kvbp.tile([128, D], BF, tag="vb")
                    for w_sb, brow, dst, act in (
                        (wk_sb, bk_sb, kt, "relu"),
                        (wv_sb, bv_sb, vt, "copy"),
                    ):
                        for hf in range(2):
                            ps = psp.tile([128, 512], f32, tag="ps")
                            for dn in range(NDIN):
                                nc.tensor.matmul(
                                    ps[:],
                                    xts[dn][:, gt * 128:(gt + 1) * 128],
                                    w_sb[dn][:, hf * 512:(hf + 1) * 512],
                                    start=(dn == 0), stop=False)
                            nc.tensor.matmul(
                                ps[:], ones[:, 0:128],
                                brow[:, hf * 512:(hf + 1) * 512],
                                start=False, stop=True)
                            dsl = dst[:, hf * 512:(hf + 1) * 512]
                            if act == "relu":
                                nc.scalar.activation(dsl, ps[:], RELU)
                            else:
                                nc.vector.tensor_copy(dsl, ps[:])
                    for p in range(NPAIR):
                        nc.tensor.matmul(
                            kvps[p // 2][:, (p % 2) * 256:(p % 2) * 256 + 256],
                            kt[:, p * 128:(p + 1) * 128],
                            vt[:, (p // 2) * 256:(p // 2) * 256 + 256],
                            start=(t == 0 and p % 2 == 0),
                            stop=(t == NTB - 1 and p % 2 == 1))
                # ship only the diagonal [64,64] blocks (head h = 2p+j)
                for p in range(NPAIR):
                    for j in range(2):
                        ex = kvexp.tile([64, 64], f32, tag="kvex",
                                        name=f"kvex{b}_{p}_{j}")
                        nc.vector.tensor_copy(
                            ex[:],
                            kvps[p // 2][j * 64:(j + 1) * 64,
                                         (p % 2) * 384 + j * 64:
                                         (p % 2) * 384 + j * 64 + 64])
                        h = 2 * p + j
                        nc.sync.dma_start(
                            bnc_in[b][h * 64:(h + 1) * 64, :], ex[:])
                nc.gpsimd.collective_compute(
                    "AllReduce", mybir.AluOpType.add,
                    replica_groups=[list(range(NCORES))],
                    ins=[bnc_in[b].opt()], outs=[bnc_out[b].opt()])

            # ---- Stage 2: Q^T projection + kv readout -> O^T ----
            ot_tiles = []
            for p in range(NPAIR):
                ot = otp.tile([128, T], BF, tag=f"ot{p}", name=f"ot{p}")
                ot_tiles.append(ot)
                for b in range(B):
                    raw = kvrawp.tile([128, 128], f32, tag="kvraw",
                                      name=f"kvraw{p}_{b}")
                    nc.vector.memset(raw[:], 0.0)
                    for j in range(2):
                        h = 2 * p + j
                        nc.sync.dma_start(
                            raw[j * 64:(j + 1) * 64, j * 64:(j + 1) * 64],
                            bnc_out[b][h * 64:(h + 1) * 64, :])
                    kvsb = kvsbp.tile([128, 128], BF, tag="kvsb",
                                      name=f"kvsb{p}_{b}")
                    nc.vector.tensor_copy(kvsb[:], raw[:])
                    ps = psp.tile([128, 512], f32, tag="ps")
                    for dn in range(NDIN):
                        nc.tensor.matmul(
                            ps[:],
                            wq_sb[dn][:, p * 128:(p + 1) * 128],
                            xts[dn][:, b * 512:(b + 1) * 512],
                            start=(dn == 0), stop=(dn == NDIN - 1))
                    qt = qtp.tile([128, 512], BF, tag="qt")
                    nc.scalar.activation(qt[:], ps[:], RELU,
                                         bias=bq_sb[:, p:p + 1])
                    pso = psp.tile([128, 512], f32, tag="ps")
                    nc.tensor.matmul(pso[:], kvsb[:], qt[:],
                                     start=True, stop=True)
                    nc.vector.tensor_copy(
                        ot[:, b * 512:(b + 1) * 512], pso[:])

            # ---- Stage 3: output projection y = O @ o_w.T + o_b ----
            for gt in range(NT):
                yt = ytp.tile([128, D], f32, tag="yt")
                for hf in range(2):
                    ps = psp.tile([128, 512], f32, tag="ps")
                    for dn in range(NDIN):
                        nc.tensor.matmul(
                            ps[:],
                            ot_tiles[dn][:, gt * 128:(gt + 1) * 128],
                            wo_sb[dn][:, hf * 512:(hf + 1) * 512],
                            start=(dn == 0), stop=False)
                    nc.tensor.matmul(
                        ps[:], ones[:, 0:128],
                        bo_sb[:, hf * 512:(hf + 1) * 512],
                        start=False, stop=True)
                    nc.vector.tensor_copy(yt[:, hf * 512:(hf + 1) * 512], ps[:])
                nc.sync.dma_start(y_d[gt * 128:(gt + 1) * 128, :], yt[:])

    nc.compile()
    _CACHE["nc_bf16"] = nc
    return nc


def prepare_in_maps(x, q_w, q_b, k_w, k_b, v_w, v_b, o_w, o_b, dtype="bf16"):
    if dtype == "bf16":
        import ml_dtypes
        mmdt = ml_dtypes.bfloat16
    else:
        mmdt = np.float32
    shared = {
        "wq": np.ascontiguousarray(q_w.T).astype(mmdt),
        "wk": np.ascontiguousarray(k_w.T).astype(mmdt),
        "wv": np.ascontiguousarray(v_w.T).astype(mmdt),
        "wo": np.ascontiguousarray(o_w.T).astype(mmdt),
        "bq": np.ascontiguousarray(q_b.reshape(NDIN, 128).T),
        "bk": k_b.reshape(1, D).astype(mmdt),
        "bv": v_b.reshape(1, D).astype(mmdt),
        "bo": o_b.reshape(1, D).astype(mmdt),
    }
    in_maps = []
    for c in range(NCORES):
        xs = x[:, c * SC:(c + 1) * SC, :].reshape(T, D)
        m = dict(shared)
        m["xt"] = np.ascontiguousarray(xs.T).astype(mmdt)
        in_maps.append(m)
    return in_maps


def gather_output(results):
    y = np.empty((B, S, D), dtype=np.float32)
    for c in range(NCORES):
        y[:, c * SC:(c + 1) * SC, :] = results[c]["y"].reshape(B, SC, D)
    return y


DTYPE = "f32r"


def run(inputs, trace=False, dtype=None, **kw):
    from concourse import bass_utils
    dtype = dtype or DTYPE
    nc = build_program_bf16() if dtype == "bf16" else build_program()
    in_maps = prepare_in_maps(**inputs, dtype=dtype)
    res = bass_utils.run_bass_kernel_spmd(
        nc, in_maps, core_ids=list(range(NCORES)), trace=trace, **kw)
    return gather_output(res.results), res


def kernel(**inputs):
    y, _ = run(inputs)
    return y



# revision 2
# speedup vs baseline: 1.3609x; 1.3609x over previous
"""Trainium2 Bass kernel for MinimalLinearAttention.

  q = relu(x @ q_w.T + q_b); k = relu(x @ k_w.T + k_b); v = x @ v_w.T + v_b
  kv[b,h] = sum_s k[b,s,h,:] outer v[b,s,h,:]          (per batch, all tokens)
  out[b,s,h] = q[b,s,h,:] @ kv[b,h]
  y = out @ o_w.T + o_b

Sharding: token-parallel over 8 cores. Each core takes a 512-token slice of
every batch (2048 tokens), computes k/v projections + partial kv, AllReduces
kv across cores (per batch, overlapped with compute), then does the q
readout + output projection for its own tokens. Host concatenates slices.

bf16 v3: all matmul operands bf16 (host-cast, no on-device converts),
every weight resident in SBUF, chunked fast-start DMA order, V bias via
DVE broadcast-add (no bias matmul), output computed as Y^T so the o-bias
fuses into the activation eviction (host transposes back), kv matmuls
software-pipelined one token-tile behind the K/V projections, and stage-2
ordered (kv loads -> q projections -> readouts -> y) per batch so the last
batch's AllReduce latency is hidden behind ~90us of compute.

On-device layouts (per core):
  xt   = x_slice.T            [D=1024, T=2048]   (T cols batch-major: b*512+s)
  wq/wk/wv/wo = W.T           [Din=1024, Dout=1024]
  K, V                        [T, D]     (from xt-stationary matmuls)
  Q^T                         [D, T]     (from w-stationary matmuls)
  kv per (batch, head-pair)   [128, 128] block-diagonal (2 heads of 64)
  O^T                         [D, T]
  Y^T                         [D, T]     (f32; host transposes)
"""

import os
import sys

os.environ.setdefault("MYCRO_LOCAL_CACHE", "1")

for _p in ("/opt/trn_rl_repo", "/root/.axon_site/_ro/trn_rl_repo"):
    if os.path.isdir(_p) and _p not in sys.path:
        sys.path.insert(0, _p)

import numpy as np

B, S, D, H, HD = 4, 4096, 1024, 16, 64
NCORES = 8
SC = S // NCORES          # 512 tokens per core per batch
T = B * SC                # 2048 rows per core
NPAIR = 8                 # head pairs (2 heads of 64 dims = 128 partitions)
NDIN = D // 128           # 8 Din tiles
NT = T // 128             # 16 T tiles per core
NTB = SC // 128           # 4 T tiles per batch

_CACHE = {}


def build_program_bf16():
    """bf16 v3 (see module docstring)."""
    if "nc_bf16" in _CACHE:
        return _CACHE["nc_bf16"]

    import concourse.bacc as bacc
    import concourse.tile as tile
    from concourse import bass, mybir

    f32 = mybir.dt.float32
    BF = mybir.dt.bfloat16
    RELU = mybir.ActivationFunctionType.Relu
    IDENT = mybir.ActivationFunctionType.Identity
    COPY = mybir.ActivationFunctionType.Copy
    ADD = mybir.AluOpType.add

    nc = bacc.Bacc("TRN2", target_bir_lowering=False, debug=False,
                   num_devices=NCORES)

    xt_d = nc.dram_tensor("xt", [D, T], BF, kind="ExternalInput").ap()
    wq_d = nc.dram_tensor("wq", [D, D], BF, kind="ExternalInput").ap()
    wk_d = nc.dram_tensor("wk", [D, D], BF, kind="ExternalInput").ap()
    wv_d = nc.dram_tensor("wv", [D, D], BF, kind="ExternalInput").ap()
    wo_d = nc.dram_tensor("wo", [D, D], BF, kind="ExternalInput").ap()
    bq_d = nc.dram_tensor("bq", [128, NDIN], f32, kind="ExternalInput").ap()
    bo_d = nc.dram_tensor("bo", [128, NDIN], f32, kind="ExternalInput").ap()
    bk_d = nc.dram_tensor("bk", [1, D], BF, kind="ExternalInput").ap()
    bv_d = nc.dram_tensor("bv", [1, D], BF, kind="ExternalInput").ap()
    y_d = nc.dram_tensor("y", [D, T], f32, kind="ExternalOutput").ap()

    HPB = 16 * 64  # bounce rows per batch: 16 heads x 64 d-rows

    with tile.TileContext(nc) as tc:
        with (
            tc.tile_pool(name="const", bufs=1) as constp,
            tc.tile_pool(name="wp", bufs=1) as wp,
            tc.tile_pool(name="xtp", bufs=1) as xtp,
            tc.tile_pool(name="kvb", bufs=3) as kvbp,
            tc.tile_pool(name="qt", bufs=2) as qtp,
            tc.tile_pool(name="otb", bufs=2) as otbp,
            tc.tile_pool(name="kvex", bufs=8) as kvexp,
            tc.tile_pool(name="kvraw", bufs=2) as kvrawp,
            tc.tile_pool(name="kvsb", bufs=2) as kvsbp,
            tc.tile_pool(name="yt", bufs=3) as ytp,
            tc.tile_pool(name="dram", bufs=1, space="DRAM") as dramp,
            tc.tile_pool(name="ps", bufs=3, space="PSUM") as psp,
            tc.tile_pool(name="pskv", bufs=4, space="PSUM") as pskvp,
        ):
            # ---- loads (program order = scheduling priority) ----
            ones = constp.tile([1, 128], BF, tag="ones")
            nc.vector.memset(ones[:], 1.0)
            bk_sb = constp.tile([1, D], BF, tag="bk")
            nc.sync.dma_start(bk_sb[:], bk_d[:])
            bv_sb = constp.tile([1, D], BF, tag="bv")
            nc.sync.dma_start(bv_sb[:], bv_d[:])

            # first-needed set: xt cols 0:512 + wk half 0 (2 MB)
            xts = []
            for dn in range(NDIN):
                t = xtp.tile([128, T], BF, tag=f"xt{dn}", name=f"xt_sb{dn}")
                nc.sync.dma_start(t[:, 0:512],
                                  xt_d[dn * 128:(dn + 1) * 128, 0:512])
                xts.append(t)
            wk_sb = []
            for dn in range(NDIN):
                t = wp.tile([128, D], BF, tag=f"wk{dn}", name=f"wk_sb{dn}")
                nc.sync.dma_start(t[:, 0:512],
                                  wk_d[dn * 128:(dn + 1) * 128, 0:512])
                wk_sb.append(t)
            for dn in range(NDIN):
                nc.sync.dma_start(wk_sb[dn][:, 512:D],
                                  wk_d[dn * 128:(dn + 1) * 128, 512:D])

            def loadw(dram_ap, tag):
                w = []
                for dn in range(NDIN):
                    t = wp.tile([128, D], BF, tag=f"{tag}{dn}",
                                name=f"{tag}_sb{dn}")
                    nc.sync.dma_start(t[:], dram_ap[dn * 128:(dn + 1) * 128, :])
                    w.append(t)
                return w

            wv_sb = loadw(wv_d, "wv")
            for c in range(1, 4):
                for dn in range(NDIN):
                    nc.sync.dma_start(
                        xts[dn][:, c * 512:(c + 1) * 512],
                        xt_d[dn * 128:(dn + 1) * 128, c * 512:(c + 1) * 512])

            # broadcast v-bias to all partitions: bvb = ones^T @ bv  (f32)
            bvb = constp.tile([128, D], f32, tag="bvb")
            for hf in range(2):
                ps = psp.tile([128, 512], f32, tag="ps")
                nc.tensor.matmul(ps[:], ones[:, 0:128],
                                 bv_sb[:, hf * 512:(hf + 1) * 512],
                                 start=True, stop=True)
                nc.scalar.activation(bvb[:, hf * 512:(hf + 1) * 512], ps[:],
                                     COPY)

            wq_sb = loadw(wq_d, "wq")
            bq_sb = constp.tile([128, NDIN], f32, tag="bq")
            nc.sync.dma_start(bq_sb[:], bq_d[:])
            wo_sb = loadw(wo_d, "wo")
            bo_sb = constp.tile([128, NDIN], f32, tag="bo")
            nc.sync.dma_start(bo_sb[:], bo_d[:])

            bnc_in = [dramp.tile([HPB, 64], f32, tag=f"bi{b}",
                                 name=f"bnc_in{b}") for b in range(B)]
            bnc_out = [dramp.tile([HPB, 64], f32, tag=f"bo{b}",
                                  addr_space="Shared", name=f"bnc_out{b}")
                       for b in range(B)]

            # ---- Stage 1: K,V projections + per-batch partial kv ----
            # kv matmuls for token-tile t are emitted after the K/V
            # projections of tile t+1 so the PE never waits on the
            # kt/vt evictions.
            for b in range(B):
                kvps = [pskvp.tile([128, 512], f32, tag="kvps",
                                   name=f"kvps{b}_{w}") for w in range(4)]
                kvq = []  # deferred kv matmuls: (t, kt, vt)

                def flush_kv(kvps=kvps):
                    t, kt, vt = kvq.pop(0)
                    for p in range(NPAIR):
                        nc.tensor.matmul(
                            kvps[p // 2][:, (p % 2) * 256:(p % 2) * 256 + 256],
                            kt[:, p * 128:(p + 1) * 128],
                            vt[:, (p // 2) * 256:(p // 2) * 256 + 256],
                            start=(t == 0 and p % 2 == 0),
                            stop=(t == NTB - 1 and p % 2 == 1))

                for t in range(NTB):
                    gt = b * NTB + t
                    kt = kvbp.tile([128, D], BF, tag="kb")
                    vt = kvbp.tile([128, D], BF, tag="vb")
                    for hf in range(2):
                        ps = psp.tile([128, 512], f32, tag="ps")
                        for dn in range(NDIN):
                            nc.tensor.matmul(
                                ps[:],
                                xts[dn][:, gt * 128:(gt + 1) * 128],
                                wk_sb[dn][:, hf * 512:(hf + 1) * 512],
                                start=(dn == 0), stop=False)
                        nc.tensor.matmul(
                            ps[:], ones[:, 0:128],
                            bk_sb[:, hf * 512:(hf + 1) * 512],
                            start=False, stop=True)
                        nc.scalar.activation(
                            kt[:, hf * 512:(hf + 1) * 512], ps[:], RELU)
                    for hf in range(2):
                        ps = psp.tile([128, 512], f32, tag="ps")
                        for dn in range(NDIN):
                            nc.tensor.matmul(
                                ps[:],
                                xts[dn][:, gt * 128:(gt + 1) * 128],
                                wv_sb[dn][:, hf * 512:(hf + 1) * 512],
                                start=(dn == 0), stop=(dn == NDIN - 1))
                        nc.vector.scalar_tensor_tensor(
                            vt[:, hf * 512:(hf + 1) * 512], ps[:], 0.0,
                            bvb[:, hf * 512:(hf + 1) * 512], ADD, ADD)
                    kvq.append((t, kt, vt))
                    if t > 0:
                        flush_kv()
                flush_kv()

                # ship only the diagonal [64,64] blocks (head h = 2p+j)
                for p in range(NPAIR):
                    for j in range(2):
                        ex = kvexp.tile([64, 64], f32, tag="kvex",
                                        name=f"kvex{b}_{p}_{j}")
                        nc.vector.tensor_copy(
                            ex[:],
                            kvps[p // 2][j * 64:(j + 1) * 64,
                                         (p % 2) * 384 + j * 64:
                                         (p % 2) * 384 + j * 64 + 64])
                        h = 2 * p + j
                        nc.sync.dma_start(
                            bnc_in[b][h * 64:(h + 1) * 64, :], ex[:])
                nc.gpsimd.collective_compute(
                    "AllReduce", mybir.AluOpType.add,
                    replica_groups=[list(range(NCORES))],
                    ins=[bnc_in[b].opt()], outs=[bnc_out[b].opt()])

            # ---- Stage 2: per batch: kv loads, Q^T proj, readout, Y^T ----
            for b in range(B):
                # kv loads first so the DMAs run behind the q projections
                kvsbs = []
                for p in range(NPAIR):
                    raw = kvrawp.tile([128, 128], f32, tag=f"kvraw{p}",
                                      name=f"kvraw{b}_{p}")
                    nc.vector.memset(raw[:], 0.0)
                    for j in range(2):
                        h = 2 * p + j
                        nc.sync.dma_start(
                            raw[j * 64:(j + 1) * 64, j * 64:(j + 1) * 64],
                            bnc_out[b][h * 64:(h + 1) * 64, :])
                    kvsb = kvsbp.tile([128, 128], BF, tag=f"kvsb{p}",
                                      name=f"kvsb{b}_{p}")
                    nc.vector.tensor_copy(kvsb[:], raw[:])
                    kvsbs.append(kvsb)

                qts = []
                for p in range(NPAIR):
                    ps = psp.tile([128, 512], f32, tag="ps")
                    for dn in range(NDIN):
                        nc.tensor.matmul(
                            ps[:],
                            wq_sb[dn][:, p * 128:(p + 1) * 128],
                            xts[dn][:, b * 512:(b + 1) * 512],
                            start=(dn == 0), stop=(dn == NDIN - 1))
                    qt = qtp.tile([128, 512], BF, tag=f"qt{p}",
                                  name=f"qt{b}_{p}")
                    nc.scalar.activation(qt[:], ps[:], RELU,
                                         bias=bq_sb[:, p:p + 1])
                    qts.append(qt)

                otbs = []
                for p in range(NPAIR):
                    pso = psp.tile([128, 512], f32, tag="ps")
                    nc.tensor.matmul(pso[:], kvsbs[p][:], qts[p][:],
                                     start=True, stop=True)
                    otb = otbp.tile([128, 512], BF, tag=f"otb{p}",
                                    name=f"otb{b}_{p}")
                    nc.vector.tensor_copy(otb[:], pso[:])
                    otbs.append(otb)

                for do in range(NDIN):
                    ps = psp.tile([128, 512], f32, tag="ps")
                    for dn in range(NDIN):
                        nc.tensor.matmul(
                            ps[:],
                            wo_sb[dn][:, do * 128:(do + 1) * 128],
                            otbs[dn][:],
                            start=(dn == 0), stop=(dn == NDIN - 1))
                    yt = ytp.tile([128, 512], f32, tag="yt")
                    nc.scalar.activation(yt[:], ps[:], IDENT,
                                         bias=bo_sb[:, do:do + 1])
                    nc.sync.dma_start(
                        y_d[do * 128:(do + 1) * 128,
                            b * 512:(b + 1) * 512], yt[:])

    nc.compile()
    _CACHE["nc_bf16"] = nc
    return nc


# test.py compatibility: the f32r build is gone; both names resolve to bf16.
def build_program():
    return build_program_bf16()


def prepare_in_maps(x, q_w, q_b, k_w, k_b, v_w, v_b, o_w, o_b, dtype="bf16"):
    import ml_dtypes
    mmdt = ml_dtypes.bfloat16
    shared = {
        "wq": np.ascontiguousarray(q_w.T).astype(mmdt),
        "wk": np.ascontiguousarray(k_w.T).astype(mmdt),
        "wv": np.ascontiguousarray(v_w.T).astype(mmdt),
        "wo": np.ascontiguousarray(o_w.T).astype(mmdt),
        "bq": np.ascontiguousarray(
            q_b.reshape(NDIN, 128).T).astype(np.float32),
        "bo": np.ascontiguousarray(
            o_b.reshape(NDIN, 128).T).astype(np.float32),
        "bk": k_b.reshape(1, D).astype(mmdt),
        "bv": v_b.reshape(1, D).astype(mmdt),
    }
    in_maps = []
    for c in range(NCORES):
        xs = x[:, c * SC:(c + 1) * SC, :].reshape(T, D)
        m = dict(shared)
        m["xt"] = np.ascontiguousarray(xs.T).astype(mmdt)
        in_maps.append(m)
    return in_maps


def gather_output(results):
    y = np.empty((B, S, D), dtype=np.float32)
    for c in range(NCORES):
        yc = results[c]["y"]
        if yc.shape == (D, T):  # Y^T layout
            yc = yc.T
        y[:, c * SC:(c + 1) * SC, :] = yc.reshape(B, SC, D)
    return y


DTYPE = "bf16"


def run(inputs, trace=False, dtype=None, **kw):
    from concourse import bass_utils
    nc = build_program_bf16()
    in_maps = prepare_in_maps(**inputs)
    res = bass_utils.run_bass_kernel_spmd(
        nc, in_maps, core_ids=list(range(NCORES)), trace=trace, **kw)
    return gather_output(res.results), res


def kernel(**inputs):
    y, _ = run(inputs)
    return y


# revision 7
# speedup vs baseline: 1.4880x; 1.0934x over previous
"""Trainium2 Bass kernel for MinimalLinearAttention.

  q = relu(x @ q_w.T + q_b); k = relu(x @ k_w.T + k_b); v = x @ v_w.T + v_b
  kv[b,h] = sum_s k[b,s,h,:] outer v[b,s,h,:]          (per batch, all tokens)
  out[b,s,h] = q[b,s,h,:] @ kv[b,h]
  y = out @ o_w.T + o_b

Sharding: token-parallel over 8 cores. Each core takes a 512-token slice of
every batch (2048 tokens), computes k/v projections + partial kv, AllReduces
kv across cores (per batch, overlapped with compute), then does the q
readout + output projection for its own tokens. Host concatenates slices.

bf16 v4: all matmul operands bf16 (host-cast), every weight resident in
SBUF, DMA order matched to the PE's consumption order, k/v biases applied
by DVE broadcast-add at PSUM eviction (no bias matmuls), per-pair kv
matmuls at N=128 into two PSUM banks, bf16 kv collective whose diagonal
blocks DMA straight into long-lived zeroed kvsb tiles (no on-device cast,
nothing in an engine stream ever waits on the collective), output computed
as Y^T so the o-bias fuses into the activation eviction (host transposes),
kv matmuls software-pipelined one token-tile behind the K/V projections,
and stage-2 ordered (kv DMAs -> q projections -> readouts -> y) per batch
so the last batch's AllReduce latency is hidden behind ~90us of compute.

On-device layouts (per core):
  xt   = x_slice.T            [D=1024, T=2048]   (T cols batch-major: b*512+s)
  wq/wk/wv/wo = W.T           [Din=1024, Dout=1024]
  K, V                        [T, D]     (from xt-stationary matmuls)
  Q^T                         [D, T]     (from w-stationary matmuls)
  kv per (batch, head-pair)   [128, 128] block-diagonal (2 heads of 64)
  O^T                         [D, T]
  Y^T                         [D, T]     (f32; host transposes)
"""

import os
import sys

os.environ.setdefault("MYCRO_LOCAL_CACHE", "1")

for _p in ("/opt/trn_rl_repo", "/root/.axon_site/_ro/trn_rl_repo"):
    if os.path.isdir(_p) and _p not in sys.path:
        sys.path.insert(0, _p)

import numpy as np

B, S, D, H, HD = 4, 4096, 1024, 16, 64
NCORES = 8
SC = S // NCORES          # 512 tokens per core per batch
T = B * SC                # 2048 rows per core
NPAIR = 8                 # head pairs (2 heads of 64 dims = 128 partitions)
NDIN = D // 128           # 8 Din tiles
NT = T // 128             # 16 T tiles per core
NTB = SC // 128           # 4 T tiles per batch

_CACHE = {}


def build_program_bf16():
    """bf16 v4 (see module docstring)."""
    if "nc_bf16" in _CACHE:
        return _CACHE["nc_bf16"]

    import concourse.bacc as bacc
    import concourse.tile as tile
    from concourse import bass, mybir

    f32 = mybir.dt.float32
    BF = mybir.dt.bfloat16
    RELU = mybir.ActivationFunctionType.Relu
    IDENT = mybir.ActivationFunctionType.Identity
    COPY = mybir.ActivationFunctionType.Copy
    ADD = mybir.AluOpType.add

    nc = bacc.Bacc("TRN2", target_bir_lowering=False, debug=False,
                   num_devices=NCORES)

    xt_d = nc.dram_tensor("xt", [D, T], BF, kind="ExternalInput").ap()
    wq_d = nc.dram_tensor("wq", [D, D], BF, kind="ExternalInput").ap()
    wk_d = nc.dram_tensor("wk", [D, D], BF, kind="ExternalInput").ap()
    wv_d = nc.dram_tensor("wv", [D, D], BF, kind="ExternalInput").ap()
    wo_d = nc.dram_tensor("wo", [D, D], BF, kind="ExternalInput").ap()
    bq_d = nc.dram_tensor("bq", [128, NDIN], f32, kind="ExternalInput").ap()
    bo_d = nc.dram_tensor("bo", [128, NDIN], f32, kind="ExternalInput").ap()
    bk_d = nc.dram_tensor("bk", [1, D], BF, kind="ExternalInput").ap()
    bv_d = nc.dram_tensor("bv", [1, D], BF, kind="ExternalInput").ap()
    y_d = nc.dram_tensor("y", [D, T], f32, kind="ExternalOutput").ap()

    HPB = 16 * 64  # bounce rows per batch: 16 heads x 64 d-rows

    with tile.TileContext(nc) as tc:
        with (
            tc.tile_pool(name="const", bufs=1) as constp,
            tc.tile_pool(name="wp", bufs=1) as wp,
            tc.tile_pool(name="xtp", bufs=1) as xtp,
            tc.tile_pool(name="kvb", bufs=3) as kvbp,
            tc.tile_pool(name="ktmp", bufs=3) as ktmpp,
            tc.tile_pool(name="qt", bufs=2) as qtp,
            tc.tile_pool(name="otb", bufs=2) as otbp,
            tc.tile_pool(name="kvex", bufs=8) as kvexp,
            tc.tile_pool(name="kvsb", bufs=1) as kvsbp,
            tc.tile_pool(name="yt", bufs=3) as ytp,
            tc.tile_pool(name="dram", bufs=1, space="DRAM") as dramp,
            tc.tile_pool(name="ps", bufs=5, space="PSUM") as psp,
            tc.tile_pool(name="pskv", bufs=2, space="PSUM") as pskvp,
        ):
            # ---- loads (program order = scheduling priority) ----
            ones = constp.tile([1, 128], BF, tag="ones")
            nc.vector.memset(ones[:], 1.0)
            bk_sb = constp.tile([1, D], BF, tag="bk")
            nc.sync.dma_start(bk_sb[:], bk_d[:])
            bv_sb = constp.tile([1, D], BF, tag="bv")
            nc.sync.dma_start(bv_sb[:], bv_d[:])

            # broadcast k/v biases to all partitions (f32) via tiny matmuls;
            # these are also the PE's warmup work during the initial DMAs
            bkb = constp.tile([128, D], f32, tag="bkb")
            bvb = constp.tile([128, D], f32, tag="bvb")
            for row, dst in ((bk_sb, bkb), (bv_sb, bvb)):
                for hf in range(2):
                    ps = psp.tile([128, 512], f32, tag="ps")
                    nc.tensor.matmul(ps[:], ones[:, 0:128],
                                     row[:, hf * 512:(hf + 1) * 512],
                                     start=True, stop=True)
                    nc.scalar.activation(dst[:, hf * 512:(hf + 1) * 512],
                                         ps[:], COPY)

            # zeroed block-diag kv holders (ping-pong per pair, stage 2)
            kvsb_pp = []
            for p in range(NPAIR):
                pair = []
                for g in range(2):
                    t = kvsbp.tile([128, 128], BF, tag=f"kvsb{p}_{g}",
                                   name=f"kvsb{p}_{g}")
                    nc.vector.memset(t[:], 0.0)
                    pair.append(t)
                kvsb_pp.append(pair)

            # first-needed set: xt cols 0:512, wk half 0, wv half 0
            xts = []
            for dn in range(NDIN):
                t = xtp.tile([128, T], BF, tag=f"xt{dn}", name=f"xt_sb{dn}")
                nc.sync.dma_start(t[:, 0:512],
                                  xt_d[dn * 128:(dn + 1) * 128, 0:512])
                xts.append(t)
            wk_sb = []
            wv_sb = []
            for w_d, lst, tag in ((wk_d, wk_sb, "wk"), (wv_d, wv_sb, "wv")):
                for dn in range(NDIN):
                    t = wp.tile([128, D], BF, tag=f"{tag}{dn}",
                                name=f"{tag}_sb{dn}")
                    nc.sync.dma_start(t[:, 0:512],
                                      w_d[dn * 128:(dn + 1) * 128, 0:512])
                    lst.append(t)
            for w_d, lst in ((wk_d, wk_sb), (wv_d, wv_sb)):
                for dn in range(NDIN):
                    nc.sync.dma_start(lst[dn][:, 512:D],
                                      w_d[dn * 128:(dn + 1) * 128, 512:D])
            for c in range(1, 4):
                for dn in range(NDIN):
                    nc.sync.dma_start(
                        xts[dn][:, c * 512:(c + 1) * 512],
                        xt_d[dn * 128:(dn + 1) * 128, c * 512:(c + 1) * 512])

            def loadw(dram_ap, tag):
                w = []
                for dn in range(NDIN):
                    t = wp.tile([128, D], BF, tag=f"{tag}{dn}",
                                name=f"{tag}_sb{dn}")
                    nc.sync.dma_start(t[:], dram_ap[dn * 128:(dn + 1) * 128, :])
                    w.append(t)
                return w

            wq_sb = loadw(wq_d, "wq")
            bq_sb = constp.tile([128, NDIN], f32, tag="bq")
            nc.sync.dma_start(bq_sb[:], bq_d[:])
            wo_sb = loadw(wo_d, "wo")
            bo_sb = constp.tile([128, NDIN], f32, tag="bo")
            nc.sync.dma_start(bo_sb[:], bo_d[:])

            bnc_in = [dramp.tile([HPB, 64], BF, tag=f"bi{b}",
                                 name=f"bnc_in{b}") for b in range(B)]
            bnc_out = [dramp.tile([HPB, 64], BF, tag=f"bo{b}",
                                  addr_space="Shared", name=f"bnc_out{b}")
                       for b in range(B)]

            # ---- Stage 1: K,V projections + per-batch partial kv ----
            # kv matmuls for token-tile t are emitted after the K/V
            # projections of tile t+1 so the PE never waits on the
            # kt/vt evictions.
            for b in range(B):
                kvps = [pskvp.tile([128, 512], f32, tag="kvps",
                                   name=f"kvps{b}_{w}") for w in range(2)]
                kvq = []  # deferred kv matmuls: (t, kt, vt)

                def flush_kv(kvps=kvps):
                    t, kt, vt = kvq.pop(0)
                    for p in range(NPAIR):
                        c0 = (p % 4) * 128
                        nc.tensor.matmul(
                            kvps[p // 4][:, c0:c0 + 128],
                            kt[:, p * 128:(p + 1) * 128],
                            vt[:, p * 128:(p + 1) * 128],
                            start=(t == 0 and p % 4 == 0),
                            stop=(t == NTB - 1 and p % 4 == 3))

                for t in range(NTB):
                    gt = b * NTB + t
                    kt = kvbp.tile([128, D], BF, tag="kb")
                    vt = kvbp.tile([128, D], BF, tag="vb")
                    for hf in range(2):
                        # K half: matmul, +bias on DVE, relu on ACT
                        ps = psp.tile([128, 512], f32, tag="ps")
                        for dn in range(NDIN):
                            nc.tensor.matmul(
                                ps[:],
                                xts[dn][:, gt * 128:(gt + 1) * 128],
                                wk_sb[dn][:, hf * 512:(hf + 1) * 512],
                                start=(dn == 0), stop=(dn == NDIN - 1))
                        ktmp = ktmpp.tile([128, 512], BF, tag="ktmp")
                        nc.vector.scalar_tensor_tensor(
                            ktmp[:], ps[:], 0.0,
                            bkb[:, hf * 512:(hf + 1) * 512], ADD, ADD)
                        nc.scalar.activation(
                            kt[:, hf * 512:(hf + 1) * 512], ktmp[:], RELU)
                        # V half: matmul, +bias on DVE
                        ps = psp.tile([128, 512], f32, tag="ps")
                        for dn in range(NDIN):
                            nc.tensor.matmul(
                                ps[:],
                                xts[dn][:, gt * 128:(gt + 1) * 128],
                                wv_sb[dn][:, hf * 512:(hf + 1) * 512],
                                start=(dn == 0), stop=(dn == NDIN - 1))
                        nc.vector.scalar_tensor_tensor(
                            vt[:, hf * 512:(hf + 1) * 512], ps[:], 0.0,
                            bvb[:, hf * 512:(hf + 1) * 512], ADD, ADD)
                    kvq.append((t, kt, vt))
                    if t > 0:
                        flush_kv()
                flush_kv()

                # ship only the diagonal [64,64] blocks (head h = 2p+j)
                for p in range(NPAIR):
                    for j in range(2):
                        ex = kvexp.tile([64, 64], BF, tag="kvex",
                                        name=f"kvex{b}_{p}_{j}")
                        c0 = (p % 4) * 128 + j * 64
                        nc.vector.tensor_copy(
                            ex[:],
                            kvps[p // 4][j * 64:(j + 1) * 64, c0:c0 + 64])
                        h = 2 * p + j
                        nc.sync.dma_start(
                            bnc_in[b][h * 64:(h + 1) * 64, :], ex[:])
                nc.gpsimd.collective_compute(
                    "AllReduce", mybir.AluOpType.add,
                    replica_groups=[list(range(NCORES))],
                    ins=[bnc_in[b].opt()], outs=[bnc_out[b].opt()])

            # ---- Stage 2: per batch: kv DMAs, Q^T proj, readout, Y^T ----
            for b in range(B):
                # diagonal kv blocks land straight in the zeroed bf16
                # holders; only these DMAs (no engine ops) wait on the
                # collective
                for p in range(NPAIR):
                    for j in range(2):
                        h = 2 * p + j
                        nc.sync.dma_start(
                            kvsb_pp[p][b % 2][j * 64:(j + 1) * 64,
                                              j * 64:(j + 1) * 64],
                            bnc_out[b][h * 64:(h + 1) * 64, :])

                qts = []
                for p in range(NPAIR):
                    ps = psp.tile([128, 512], f32, tag="ps")
                    for dn in range(NDIN):
                        nc.tensor.matmul(
                            ps[:],
                            wq_sb[dn][:, p * 128:(p + 1) * 128],
                            xts[dn][:, b * 512:(b + 1) * 512],
                            start=(dn == 0), stop=(dn == NDIN - 1))
                    qt = qtp.tile([128, 512], BF, tag=f"qt{p}",
                                  name=f"qt{b}_{p}")
                    nc.scalar.activation(qt[:], ps[:], RELU,
                                         bias=bq_sb[:, p:p + 1])
                    qts.append(qt)

                otbs = []
                for p in range(NPAIR):
                    pso = psp.tile([128, 512], f32, tag="ps")
                    nc.tensor.matmul(pso[:], kvsb_pp[p][b % 2][:], qts[p][:],
                                     start=True, stop=True)
                    otb = otbp.tile([128, 512], BF, tag=f"otb{p}",
                                    name=f"otb{b}_{p}")
                    nc.vector.tensor_copy(otb[:], pso[:])
                    otbs.append(otb)

                for do in range(NDIN):
                    ps = psp.tile([128, 512], f32, tag="ps")
                    for dn in range(NDIN):
                        nc.tensor.matmul(
                            ps[:],
                            wo_sb[dn][:, do * 128:(do + 1) * 128],
                            otbs[dn][:],
                            start=(dn == 0), stop=(dn == NDIN - 1))
                    yt = ytp.tile([128, 512], f32, tag="yt")
                    nc.scalar.activation(yt[:], ps[:], IDENT,
                                         bias=bo_sb[:, do:do + 1])
                    nc.sync.dma_start(
                        y_d[do * 128:(do + 1) * 128,
                            b * 512:(b + 1) * 512], yt[:])

    nc.compile()
    _CACHE["nc_bf16"] = nc
    return nc


# test.py compatibility: both names resolve to the bf16 build.
def build_program():
    return build_program_bf16()


def prepare_in_maps(x, q_w, q_b, k_w, k_b, v_w, v_b, o_w, o_b, dtype="bf16"):
    import ml_dtypes
    mmdt = ml_dtypes.bfloat16
    shared = {
        "wq": np.ascontiguousarray(q_w.T).astype(mmdt),
        "wk": np.ascontiguousarray(k_w.T).astype(mmdt),
        "wv": np.ascontiguousarray(v_w.T).astype(mmdt),
        "wo": np.ascontiguousarray(o_w.T).astype(mmdt),
        "bq": np.ascontiguousarray(
            q_b.reshape(NDIN, 128).T).astype(np.float32),
        "bo": np.ascontiguousarray(
            o_b.reshape(NDIN, 128).T).astype(np.float32),
        "bk": k_b.reshape(1, D).astype(mmdt),
        "bv": v_b.reshape(1, D).astype(mmdt),
    }
    in_maps = []
    for c in range(NCORES):
        xs = x[:, c * SC:(c + 1) * SC, :].reshape(T, D)
        m = dict(shared)
        m["xt"] = np.ascontiguousarray(xs.T).astype(mmdt)
        in_maps.append(m)
    return in_maps


def gather_output(results):
    y = np.empty((B, S, D), dtype=np.float32)
    for c in range(NCORES):
        yc = results[c]["y"]
        if yc.shape == (D, T):  # Y^T layout
            yc = yc.T
        y[:, c * SC:(c + 1) * SC, :] = yc.reshape(B, SC, D)
    return y


DTYPE = "bf16"


def run(inputs, trace=False, dtype=None, **kw):
    from concourse import bass_utils
    nc = build_program_bf16()
    in_maps = prepare_in_maps(**inputs)
    res = bass_utils.run_bass_kernel_spmd(
        nc, in_maps, core_ids=list(range(NCORES)), trace=trace, **kw)
    return gather_output(res.results), res


def kernel(**inputs):
    y, _ = run(inputs)
    return y


# revision 11
# speedup vs baseline: 1.5278x; 1.0267x over previous
"""Trainium2 Bass kernel for MinimalLinearAttention.

  q = relu(x @ q_w.T + q_b); k = relu(x @ k_w.T + k_b); v = x @ v_w.T + v_b
  kv[b,h] = sum_s k[b,s,h,:] outer v[b,s,h,:]          (per batch, all tokens)
  out[b,s,h] = q[b,s,h,:] @ kv[b,h]
  y = out @ o_w.T + o_b

Sharding: token-parallel over 8 cores. Each core takes a 512-token slice of
every batch (2048 tokens), computes k/v projections + partial kv, AllReduces
kv across cores (per batch, overlapped with compute), then does the q
readout + output projection for its own tokens. Host concatenates slices.

bf16 v4: all matmul operands bf16 (host-cast), every weight resident in
SBUF, DMA order matched to the PE's consumption order, k/v biases applied
by DVE broadcast-add at PSUM eviction (no bias matmuls), per-pair kv
matmuls at N=128 into two PSUM banks, bf16 kv collective whose diagonal
blocks DMA straight into long-lived zeroed kvsb tiles (no on-device cast,
nothing in an engine stream ever waits on the collective), output computed
as Y^T so the o-bias fuses into the activation eviction (host transposes),
kv matmuls software-pipelined one token-tile behind the K/V projections,
and stage-2 ordered (kv DMAs -> q projections -> readouts -> y) per batch
so the last batch's AllReduce latency is hidden behind ~90us of compute.

On-device layouts (per core):
  xt   = x_slice.T            [D=1024, T=2048]   (T cols batch-major: b*512+s)
  wq/wk/wv/wo = W.T           [Din=1024, Dout=1024]
  K, V                        [T, D]     (from xt-stationary matmuls)
  Q^T                         [D, T]     (from w-stationary matmuls)
  kv per (batch, head-pair)   [128, 128] block-diagonal (2 heads of 64)
  O^T                         [D, T]
  Y^T                         [D, T]     (f32; host transposes)
"""

import os
import sys

os.environ.setdefault("MYCRO_LOCAL_CACHE", "1")

for _p in ("/opt/trn_rl_repo", "/root/.axon_site/_ro/trn_rl_repo"):
    if os.path.isdir(_p) and _p not in sys.path:
        sys.path.insert(0, _p)

import numpy as np

B, S, D, H, HD = 4, 4096, 1024, 16, 64
NCORES = 8
SC = S // NCORES          # 512 tokens per core per batch
T = B * SC                # 2048 rows per core
NPAIR = 8                 # head pairs (2 heads of 64 dims = 128 partitions)
NDIN = D // 128           # 8 Din tiles
NT = T // 128             # 16 T tiles per core
NTB = SC // 128           # 4 T tiles per batch

_CACHE = {}


def build_program_bf16():
    """bf16 v4 (see module docstring)."""
    if "nc_bf16" in _CACHE:
        return _CACHE["nc_bf16"]

    import concourse.bacc as bacc
    import concourse.tile as tile
    from concourse import bass, mybir

    f32 = mybir.dt.float32
    BF = mybir.dt.bfloat16
    RELU = mybir.ActivationFunctionType.Relu
    IDENT = mybir.ActivationFunctionType.Identity
    COPY = mybir.ActivationFunctionType.Copy
    ADD = mybir.AluOpType.add

    nc = bacc.Bacc("TRN2", target_bir_lowering=False, debug=False,
                   num_devices=NCORES)

    xt_d = nc.dram_tensor("xt", [D, T], BF, kind="ExternalInput").ap()
    wq_d = nc.dram_tensor("wq", [D, D], BF, kind="ExternalInput").ap()
    wk_d = nc.dram_tensor("wk", [D, D], BF, kind="ExternalInput").ap()
    wv_d = nc.dram_tensor("wv", [D, D], BF, kind="ExternalInput").ap()
    wo_d = nc.dram_tensor("wo", [D, D], BF, kind="ExternalInput").ap()
    bq_d = nc.dram_tensor("bq", [128, NDIN], f32, kind="ExternalInput").ap()
    bo_d = nc.dram_tensor("bo", [128, NDIN], f32, kind="ExternalInput").ap()
    bk_d = nc.dram_tensor("bk", [1, D], BF, kind="ExternalInput").ap()
    bv_d = nc.dram_tensor("bv", [1, D], BF, kind="ExternalInput").ap()
    y_d = nc.dram_tensor("y", [D, T], f32, kind="ExternalOutput").ap()

    HPB = 16 * 64  # bounce rows per batch: 16 heads x 64 d-rows

    with tile.TileContext(nc) as tc:
        with (
            tc.tile_pool(name="const", bufs=1) as constp,
            tc.tile_pool(name="wp", bufs=1) as wp,
            tc.tile_pool(name="xtp", bufs=1) as xtp,
            tc.tile_pool(name="kvb", bufs=1) as kvbp,
            tc.tile_pool(name="ktmp", bufs=3) as ktmpp,
            tc.tile_pool(name="qt", bufs=2) as qtp,
            tc.tile_pool(name="otb", bufs=2) as otbp,
            tc.tile_pool(name="kvex", bufs=8) as kvexp,
            tc.tile_pool(name="kvsb", bufs=1) as kvsbp,
            tc.tile_pool(name="yt", bufs=3) as ytp,
            tc.tile_pool(name="dram", bufs=1, space="DRAM") as dramp,
            tc.tile_pool(name="ps", bufs=5, space="PSUM") as psp,
            tc.tile_pool(name="pskv", bufs=2, space="PSUM") as pskvp,
        ):
            # ---- loads (program order = scheduling priority) ----
            ones = constp.tile([1, 128], BF, tag="ones")
            nc.vector.memset(ones[:], 1.0)
            bk_sb = constp.tile([1, D], BF, tag="bk")
            nc.sync.dma_start(bk_sb[:], bk_d[:])
            bv_sb = constp.tile([1, D], BF, tag="bv")
            nc.sync.dma_start(bv_sb[:], bv_d[:])

            # PE warm-up: cheap dummy matmuls keep the PE continuously busy
            # through the initial DMA window so the HAM clock-gate releases
            # (1.2 -> 2.4 GHz) before the real chains start
            warm = pskvp.tile([128, 512], f32, tag="kvps", name="warm")
            for i in range(48):
                nc.tensor.matmul(warm[:, 0:128], ones[:, 0:128],
                                 bk_sb[:, 0:128], start=True, stop=True)

            # broadcast k/v biases to all partitions (f32) via tiny matmuls
            bkb = constp.tile([128, D], f32, tag="bkb")
            bvb = constp.tile([128, D], f32, tag="bvb")
            for row, dst in ((bk_sb, bkb), (bv_sb, bvb)):
                for hf in range(2):
                    ps = psp.tile([128, 512], f32, tag="ps")
                    nc.tensor.matmul(ps[:], ones[:, 0:128],
                                     row[:, hf * 512:(hf + 1) * 512],
                                     start=True, stop=True)
                    nc.scalar.activation(dst[:, hf * 512:(hf + 1) * 512],
                                         ps[:], COPY)

            # zeroed block-diag kv holders (ping-pong per pair, stage 2)
            kvsb_pp = []
            for p in range(NPAIR):
                pair = []
                for g in range(2):
                    t = kvsbp.tile([128, 128], BF, tag=f"kvsb{p}_{g}",
                                   name=f"kvsb{p}_{g}")
                    nc.vector.memset(t[:], 0.0)
                    pair.append(t)
                kvsb_pp.append(pair)

            # first-needed set: xt cols 0:512, wk half 0, wv half 0
            xts = []
            for dn in range(NDIN):
                t = xtp.tile([128, T], BF, tag=f"xt{dn}", name=f"xt_sb{dn}")
                nc.sync.dma_start(t[:, 0:512],
                                  xt_d[dn * 128:(dn + 1) * 128, 0:512])
                xts.append(t)
            wk_sb = []
            wv_sb = []
            for w_d, lst, tag in ((wk_d, wk_sb, "wk"), (wv_d, wv_sb, "wv")):
                for dn in range(NDIN):
                    t = wp.tile([128, D], BF, tag=f"{tag}{dn}",
                                name=f"{tag}_sb{dn}")
                    nc.sync.dma_start(t[:, 0:512],
                                      w_d[dn * 128:(dn + 1) * 128, 0:512])
                    lst.append(t)
            for w_d, lst in ((wk_d, wk_sb), (wv_d, wv_sb)):
                for dn in range(NDIN):
                    nc.sync.dma_start(lst[dn][:, 512:D],
                                      w_d[dn * 128:(dn + 1) * 128, 512:D])
            for c in range(1, 4):
                for dn in range(NDIN):
                    nc.sync.dma_start(
                        xts[dn][:, c * 512:(c + 1) * 512],
                        xt_d[dn * 128:(dn + 1) * 128, c * 512:(c + 1) * 512])

            def loadw(dram_ap, tag):
                w = []
                for dn in range(NDIN):
                    t = wp.tile([128, D], BF, tag=f"{tag}{dn}",
                                name=f"{tag}_sb{dn}")
                    nc.sync.dma_start(t[:], dram_ap[dn * 128:(dn + 1) * 128, :])
                    w.append(t)
                return w

            wq_sb = loadw(wq_d, "wq")
            bq_sb = constp.tile([128, NDIN], f32, tag="bq")
            nc.sync.dma_start(bq_sb[:], bq_d[:])
            wo_sb = loadw(wo_d, "wo")
            bo_sb = constp.tile([128, NDIN], f32, tag="bo")
            nc.sync.dma_start(bo_sb[:], bo_d[:])

            bnc_in = [dramp.tile([HPB, 64], BF, tag=f"bi{b}",
                                 name=f"bnc_in{b}") for b in range(B)]
            bnc_out = [dramp.tile([HPB, 64], BF, tag=f"bo{b}",
                                  addr_space="Shared", name=f"bnc_out{b}")
                       for b in range(B)]

            # ---- Stage 1: K,V projections + per-batch partial kv ----
            # Sweep order (all K-h0 chains, then V-h0, K-h1, V-h1, then the
            # kv matmuls): the first sweep depends only on xt-c0 + wk-h0 so
            # the PE streams without DMA stalls from the very start, and
            # evictions always finish well before the kv sweep reads them.
            for b in range(B):
                kvps = [pskvp.tile([128, 512], f32, tag="kvps",
                                   name=f"kvps{b}_{w}") for w in range(2)]
                kts = [kvbp.tile([128, D], BF, tag=f"kb{t}",
                                 name=f"kt{b}_{t}") for t in range(NTB)]
                vts = [kvbp.tile([128, D], BF, tag=f"vb{t}",
                                 name=f"vt{b}_{t}") for t in range(NTB)]

                for hf in range(2):
                    for t in range(NTB):  # K half-sweep
                        gt = b * NTB + t
                        ps = psp.tile([128, 512], f32, tag="ps")
                        for dn in range(NDIN):
                            nc.tensor.matmul(
                                ps[:],
                                xts[dn][:, gt * 128:(gt + 1) * 128],
                                wk_sb[dn][:, hf * 512:(hf + 1) * 512],
                                start=(dn == 0), stop=(dn == NDIN - 1))
                        ktmp = ktmpp.tile([128, 512], BF, tag="ktmp")
                        nc.vector.scalar_tensor_tensor(
                            ktmp[:], ps[:], 0.0,
                            bkb[:, hf * 512:(hf + 1) * 512], ADD, ADD)
                        nc.scalar.activation(
                            kts[t][:, hf * 512:(hf + 1) * 512], ktmp[:], RELU)
                    for t in range(NTB):  # V half-sweep
                        gt = b * NTB + t
                        ps = psp.tile([128, 512], f32, tag="ps")
                        for dn in range(NDIN):
                            nc.tensor.matmul(
                                ps[:],
                                xts[dn][:, gt * 128:(gt + 1) * 128],
                                wv_sb[dn][:, hf * 512:(hf + 1) * 512],
                                start=(dn == 0), stop=(dn == NDIN - 1))
                        nc.vector.scalar_tensor_tensor(
                            vts[t][:, hf * 512:(hf + 1) * 512], ps[:], 0.0,
                            bvb[:, hf * 512:(hf + 1) * 512], ADD, ADD)

                for t in range(NTB):  # kv sweep
                    for p in range(NPAIR):
                        c0 = (p % 4) * 128
                        nc.tensor.matmul(
                            kvps[p // 4][:, c0:c0 + 128],
                            kts[t][:, p * 128:(p + 1) * 128],
                            vts[t][:, p * 128:(p + 1) * 128],
                            start=(t == 0 and p % 4 == 0),
                            stop=(t == NTB - 1 and p % 4 == 3))

                # ship only the diagonal [64,64] blocks (head h = 2p+j)
                for p in range(NPAIR):
                    for j in range(2):
                        ex = kvexp.tile([64, 64], BF, tag="kvex",
                                        name=f"kvex{b}_{p}_{j}")
                        c0 = (p % 4) * 128 + j * 64
                        nc.vector.tensor_copy(
                            ex[:],
                            kvps[p // 4][j * 64:(j + 1) * 64, c0:c0 + 64])
                        h = 2 * p + j
                        nc.sync.dma_start(
                            bnc_in[b][h * 64:(h + 1) * 64, :], ex[:])
                nc.gpsimd.collective_compute(
                    "AllReduce", mybir.AluOpType.add,
                    replica_groups=[list(range(NCORES))],
                    ins=[bnc_in[b].opt()], outs=[bnc_out[b].opt()])

            # ---- Stage 2: per batch: kv DMAs, Q^T proj, readout, Y^T ----
            for b in range(B):
                # diagonal kv blocks land straight in the zeroed bf16
                # holders; only these DMAs (no engine ops) wait on the
                # collective
                for p in range(NPAIR):
                    for j in range(2):
                        h = 2 * p + j
                        nc.sync.dma_start(
                            kvsb_pp[p][b % 2][j * 64:(j + 1) * 64,
                                              j * 64:(j + 1) * 64],
                            bnc_out[b][h * 64:(h + 1) * 64, :])

                qts = []
                for p in range(NPAIR):
                    ps = psp.tile([128, 512], f32, tag="ps")
                    for dn in range(NDIN):
                        nc.tensor.matmul(
                            ps[:],
                            wq_sb[dn][:, p * 128:(p + 1) * 128],
                            xts[dn][:, b * 512:(b + 1) * 512],
                            start=(dn == 0), stop=(dn == NDIN - 1))
                    qt = qtp.tile([128, 512], BF, tag=f"qt{p}",
                                  name=f"qt{b}_{p}")
                    nc.scalar.activation(qt[:], ps[:], RELU,
                                         bias=bq_sb[:, p:p + 1])
                    qts.append(qt)

                otbs = []
                for p in range(NPAIR):
                    pso = psp.tile([128, 512], f32, tag="ps")
                    nc.tensor.matmul(pso[:], kvsb_pp[p][b % 2][:], qts[p][:],
                                     start=True, stop=True)
                    otb = otbp.tile([128, 512], BF, tag=f"otb{p}",
                                    name=f"otb{b}_{p}")
                    nc.vector.tensor_copy(otb[:], pso[:])
                    otbs.append(otb)

                for do in range(NDIN):
                    ps = psp.tile([128, 512], f32, tag="ps")
                    for dn in range(NDIN):
                        nc.tensor.matmul(
                            ps[:],
                            wo_sb[dn][:, do * 128:(do + 1) * 128],
                            otbs[dn][:],
                            start=(dn == 0), stop=(dn == NDIN - 1))
                    yt = ytp.tile([128, 512], f32, tag="yt")
                    nc.scalar.activation(yt[:], ps[:], IDENT,
                                         bias=bo_sb[:, do:do + 1])
                    nc.sync.dma_start(
                        y_d[do * 128:(do + 1) * 128,
                            b * 512:(b + 1) * 512], yt[:])

    nc.compile()
    _CACHE["nc_bf16"] = nc
    return nc


# test.py compatibility: both names resolve to the bf16 build.
def build_program():
    return build_program_bf16()


def prepare_in_maps(x, q_w, q_b, k_w, k_b, v_w, v_b, o_w, o_b, dtype="bf16"):
    import ml_dtypes
    mmdt = ml_dtypes.bfloat16
    shared = {
        "wq": np.ascontiguousarray(q_w.T).astype(mmdt),
        "wk": np.ascontiguousarray(k_w.T).astype(mmdt),
        "wv": np.ascontiguousarray(v_w.T).astype(mmdt),
        "wo": np.ascontiguousarray(o_w.T).astype(mmdt),
        "bq": np.ascontiguousarray(
            q_b.reshape(NDIN, 128).T).astype(np.float32),
        "bo": np.ascontiguousarray(
            o_b.reshape(NDIN, 128).T).astype(np.float32),
        "bk": k_b.reshape(1, D).astype(mmdt),
        "bv": v_b.reshape(1, D).astype(mmdt),
    }
    in_maps = []
    for c in range(NCORES):
        xs = x[:, c * SC:(c + 1) * SC, :].reshape(T, D)
        m = dict(shared)
        m["xt"] = np.ascontiguousarray(xs.T).astype(mmdt)
        in_maps.append(m)
    return in_maps


def gather_output(results):
    y = np.empty((B, S, D), dtype=np.float32)
    for c in range(NCORES):
        yc = results[c]["y"]
        if yc.shape == (D, T):  # Y^T layout
            yc = yc.T
        y[:, c * SC:(c + 1) * SC, :] = yc.reshape(B, SC, D)
    return y


DTYPE = "bf16"


def run(inputs, trace=False, dtype=None, **kw):
    from concourse import bass_utils
    nc = build_program_bf16()
    in_maps = prepare_in_maps(**inputs)
    res = bass_utils.run_bass_kernel_spmd(
        nc, in_maps, core_ids=list(range(NCORES)), trace=trace, **kw)
    return gather_output(res.results), res


def kernel(**inputs):
    y, _ = run(inputs)
    return y
